# revision 1
# baseline (speedup 1.0000x reference)
"""Distributed 2-layer GCN (DGL GraphConv x2 + ReLU) on 8 Trainium2 NeuronCores.

Strategy (1D dst-node partitioning):
  - Core k owns dst nodes [k*12500, (k+1)*12500). Host buckets edges by dst
    partition, sorts by dst block (128 dst nodes per block), and splits each
    block's edges by src range into 4 buckets of 25000 nodes (dma_gather uses
    int16 indices, so gathers are relative to a bucket's table base).
  - Chunk = 128 edges. Quotas (chunks per (group, bucket)) are max-reduced
    over cores/blocks so the SPMD instruction stream is identical on all
    cores; slack slots gather row 0 of the bucket with an all-zero one-hot.
  - Layer 1: dma_gather h_pre[src] rows (h_pre = h * out_norm, bf16) +
    one-hot matmul segment-sum into PSUM (aggT[f,d]), * in_norm, @W1, +b1,
    relu, @W2, * out_norm -> p2 shard (fp32).
  - AllGather p2 shards across the 8 cores.
  - Layer 2: dma_gather p2_full[src] rows + one-hot matmul segment-sum,
    * in_norm, + b2 -> output shard.
"""

import numpy as np
import ml_dtypes

N, E, IN, HID, OUT = 100000, 1600000, 128, 256, 64
NCORES = 8
NLOC = N // NCORES            # 12500
P = 128
NBLK = (NLOC + P - 1) // P    # 98
LAST_ROWS = NLOC - (NBLK - 1) * P  # 84
BF16 = ml_dtypes.bfloat16
NBUCK = 4
BUCK = 25000                  # bucket size (int16-safe)
GB = 4                        # dst-blocks per gather group
SUBMAX = 14                   # max chunks per dma_gather (SWDGE ring: <=121 descs)


def _host_prep(h, src, dst, W1, b1, W2, b2):
    deg_in = np.bincount(dst, minlength=N)
    deg_out = np.bincount(src, minlength=N)
    nin = (np.clip(deg_in, 1.0, None) ** -0.5).astype(np.float32)
    nout = (np.clip(deg_out, 1.0, None) ** -0.5).astype(np.float32)

    hpre = (h.astype(np.float32) * nout[:, None]).astype(BF16)

    ngrp = -(-NBLK // GB)
    # per core: edges sorted by (block, bucket); counts[core, blk, t]
    counts = np.zeros((NCORES, NBLK, NBUCK), np.int64)
    edges = []
    for k in range(NCORES):
        sel = (dst // NLOC) == k
        es = src[sel].astype(np.int64)
        ed = (dst[sel] - k * NLOC).astype(np.int64)
        key = (ed // P) * NBUCK + (es // BUCK)
        order = np.argsort(key, kind="stable")
        es, ed, key = es[order], ed[order], key[order]
        c = np.bincount(key, minlength=NBLK * NBUCK)
        counts[k] = c.reshape(NBLK, NBUCK)
        edges.append((es, ed))

    # quota per (group, bucket): max over cores and blocks-in-group, chunks
    Q = np.zeros((ngrp, NBUCK), np.int64)
    for g in range(ngrp):
        b0, b1_ = g * GB, min((g + 1) * GB, NBLK)
        Q[g] = np.maximum(1, -(-counts[:, b0:b1_, :].max(axis=(0, 1)) // P))

    # global chunk layout: g -> t -> block-in-group -> c
    # chunk columns and slot offsets are identical on every core
    grp_nb = [min(GB, NBLK - g * GB) for g in range(ngrp)]
    total_chunks = int(sum(grp_nb[g] * Q[g].sum() for g in range(ngrp)))
    TOT = total_chunks * P

    in_maps = []
    for k in range(NCORES):
        es, ed = edges[k]
        idx = np.zeros(TOT, np.int32)          # bucket-relative src index
        dstloc = np.full(TOT, 999.0, np.float32)
        starts = np.concatenate(
            [[0], np.cumsum(counts[k].reshape(-1))]
        ).astype(np.int64)
        pos = 0
        for g in range(ngrp):
            for t in range(NBUCK):
                for bl in range(grp_nb[g]):
                    b = g * GB + bl
                    n_bt = int(counts[k, b, t])
                    s = int(starts[b * NBUCK + t])
                    slot = pos
                    idx[slot : slot + n_bt] = es[s : s + n_bt] - t * BUCK
                    dstloc[slot : slot + n_bt] = (ed[s : s + n_bt] % P)
                    pos += int(Q[g, t]) * P
        assert pos == TOT

        # wrap idx int16: slot j of each gather at [j%16, j//16]; one gather
        # covers a contiguous slot range that is a multiple of 128, so a
        # single global 16-wrap works for every slice.
        wrap = np.tile(idx.astype(np.int16).reshape(-1, 16).T, (8, 1))

        nin_loc = nin[k * NLOC : (k + 1) * NLOC]
        nout_loc = nout[k * NLOC : (k + 1) * NLOC]
        pad = NBLK * P - NLOC
        nin_cols = np.ascontiguousarray(
            np.pad(nin_loc, (0, pad)).reshape(NBLK, P).T, dtype=np.float32)
        nout_cols = np.ascontiguousarray(
            np.pad(nout_loc, (0, pad)).reshape(NBLK, P).T, dtype=np.float32)
        nin_tiled = np.tile(np.pad(nin_loc, (0, pad)), (P, 1)).astype(np.float32)

        in_maps.append({
            "hpre": hpre,
            "idx": np.ascontiguousarray(wrap),                     # [128, TOT/16] i16
            "dstloc": np.ascontiguousarray(
                dstloc.reshape(-1, P).T).astype(BF16),             # [128, TOTCH]
            "iota": np.tile(np.arange(P, dtype=np.float32),
                            (P, 1)).astype(BF16),
            "w1": np.ascontiguousarray(W1, dtype=np.float32),
            "w2p": np.ascontiguousarray(
                W2.reshape(2, P, OUT).transpose(1, 0, 2).reshape(P, 2 * OUT),
                dtype=np.float32),
            "b1p": np.ascontiguousarray(b1.reshape(2, P).T, dtype=np.float32),
            "b2bc": np.tile(b2.astype(np.float32), (P, 1)),
            "nin_tiled": nin_tiled,
            "nin_cols": nin_cols,
            "nout_cols": nout_cols,
        })
    return Q, total_chunks, in_maps


def _build_program(Q):
    import concourse.bacc as bacc
    import concourse.mybir as mybir
    import concourse.tile as tile

    f32 = mybir.dt.float32
    bf16 = mybir.dt.bfloat16
    i16 = mybir.dt.int16

    ngrp = Q.shape[0]
    grp_nb = [min(GB, NBLK - g * GB) for g in range(ngrp)]
    total_chunks = int(sum(grp_nb[g] * Q[g].sum() for g in range(ngrp)))
    TOT = total_chunks * P
    maxq = int(Q.max())
    max_slab = GB * maxq          # chunks in the largest (group, bucket) slab

    nc = bacc.Bacc(None)

    def subsplit(nch):
        """Split nch chunks into sub-gather chunk counts <= SUBMAX."""
        nsub = -(-nch // SUBMAX)
        base = nch // nsub
        rem = nch - base * nsub
        return [base + (1 if i < rem else 0) for i in range(nsub)]

    # one register per distinct gather size
    sizes = set()
    for g in range(ngrp):
        for t in range(NBUCK):
            for s in subsplit(grp_nb[g] * int(Q[g, t])):
                sizes.add(s * P)
    sizes = sorted(sizes)
    assert len(sizes) <= 24, sizes
    size_regs = {s: nc.gpsimd.to_reg(s) for s in sizes}

    hpre_d = nc.dram_tensor("hpre", [N, IN], bf16, kind="ExternalInput")
    idx_d = nc.dram_tensor("idx", [P, TOT // 16], i16, kind="ExternalInput")
    dstloc_d = nc.dram_tensor("dstloc", [P, total_chunks], bf16, kind="ExternalInput")
    iota_d = nc.dram_tensor("iota", [P, P], bf16, kind="ExternalInput")
    w1_d = nc.dram_tensor("w1", [IN, HID], f32, kind="ExternalInput")
    w2p_d = nc.dram_tensor("w2p", [P, 2 * OUT], f32, kind="ExternalInput")
    b1p_d = nc.dram_tensor("b1p", [P, 2], f32, kind="ExternalInput")
    b2bc_d = nc.dram_tensor("b2bc", [P, OUT], f32, kind="ExternalInput")
    ninT_d = nc.dram_tensor("nin_tiled", [P, NBLK * P], f32, kind="ExternalInput")
    nincol_d = nc.dram_tensor("nin_cols", [P, NBLK], f32, kind="ExternalInput")
    noutcol_d = nc.dram_tensor("nout_cols", [P, NBLK], f32, kind="ExternalInput")
    out_d = nc.dram_tensor("out", [NLOC, OUT], f32, kind="ExternalOutput")

    with tile.TileContext(nc) as tc:
        with (
            tc.tile_pool(name="const", bufs=1) as constp,
            tc.tile_pool(name="dram", bufs=1, space="DRAM") as dramp,
            tc.tile_pool(name="x1", bufs=3) as x1p,
            tc.tile_pool(name="x2", bufs=3) as x2p,
            tc.tile_pool(name="mblk", bufs=3) as mp,
            tc.tile_pool(name="work", bufs=3) as wp,
            tc.tile_pool(name="pa", bufs=GB, space="PSUM") as pap,
            tc.tile_pool(name="pz", bufs=2, space="PSUM") as pzp,
            tc.tile_pool(name="pp", bufs=2, space="PSUM") as ppp,
        ):
            idx_sb = constp.tile([P, TOT // 16], i16, tag="idx")
            nc.sync.dma_start(out=idx_sb[:], in_=idx_d[:])
            dstloc_sb = constp.tile([P, total_chunks], bf16, tag="dstloc")
            nc.sync.dma_start(out=dstloc_sb[:], in_=dstloc_d[:])
            iotaw_sb = constp.tile([P, max_slab * P], bf16, tag="iotaw")
            for c in range(max_slab):
                nc.sync.dma_start(out=iotaw_sb[:, c * P : (c + 1) * P], in_=iota_d[:])
            w1_sb = constp.tile([IN, HID], f32, tag="w1")
            nc.sync.dma_start(out=w1_sb[:], in_=w1_d[:])
            w2_sb = constp.tile([P, 2 * OUT], f32, tag="w2")
            nc.sync.dma_start(out=w2_sb[:], in_=w2p_d[:])
            b1_sb = constp.tile([P, 2], f32, tag="b1")
            nc.sync.dma_start(out=b1_sb[:], in_=b1p_d[:])
            b2_sb = constp.tile([P, OUT], f32, tag="b2")
            nc.sync.dma_start(out=b2_sb[:], in_=b2bc_d[:])
            ninT_sb = constp.tile([P, NBLK * P], f32, tag="ninT")
            nc.sync.dma_start(out=ninT_sb[:], in_=ninT_d[:])
            nincol_sb = constp.tile([P, NBLK], f32, tag="nincol")
            nc.sync.dma_start(out=nincol_sb[:], in_=nincol_d[:])
            noutcol_sb = constp.tile([P, NBLK], f32, tag="noutcol")
            nc.sync.dma_start(out=noutcol_sb[:], in_=noutcol_d[:])

            # p2 stored bf16 padded to 128 cols: 256B rows (dma_gather
            # needs elem_size % 256B == 0); pad half is never read.
            p2_shard = dramp.tile([NLOC, 2 * OUT], bf16, tag="p2s")
            p2_full = dramp.tile([N, 2 * OUT], bf16, tag="p2f")

            def scatter_layer(layer):
                """Common gather + one-hot scatter structure for both layers."""
                elem = IN if layer == 1 else 2 * OUT
                table = hpre_d if layer == 1 else p2_full
                xpool = x1p if layer == 1 else x2p
                xdt = bf16
                slot0 = 0
                ch0 = 0
                for g in range(ngrp):
                    nb = grp_nb[g]
                    b0 = g * GB
                    aggs_psum = [
                        pap.tile([P, P if layer == 1 else OUT], f32,
                                 tag="pa", name=f"pa_l{layer}_g{g}_{bl}")
                        for bl in range(nb)
                    ]
                    qsum = int(Q[g].sum())
                    ch_base = ch0
                    sl_base = slot0
                    for t in range(NBUCK):
                        nch = nb * int(Q[g, t])
                        ni = nch * P
                        xg = xpool.tile([P, max_slab * elem], xdt, tag="xg")
                        tab_ap = table[t * BUCK : (t + 1) * BUCK, :]
                        sub0 = 0
                        for snch in subsplit(nch):
                            sni = snch * P
                            sb0 = sl_base + sub0 * P
                            nc.gpsimd.dma_gather(
                                out_ap=xg[:, sub0 * elem : (sub0 + snch) * elem]
                                    .rearrange("p (c d) -> p c d", d=elem),
                                in_ap=tab_ap,
                                idxs_ap=idx_sb[:, sb0 // 16 : (sb0 + sni) // 16],
                                num_idxs=sni,
                                num_idxs_reg=size_regs[sni],
                                elem_size=elem,
                                single_packet=False,
                            )
                            sub0 += snch
                        mb = mp.tile([P, max_slab * P], bf16, tag="m")
                        dl = dstloc_sb[:, ch_base : ch_base + nch]
                        nc.vector.tensor_tensor(
                            out=mb[:, : nch * P].rearrange("p (c d) -> p c d", d=P),
                            in0=iotaw_sb[:, : nch * P].rearrange(
                                "p (c d) -> p c d", d=P),
                            in1=dl.rearrange("p (c one) -> p c one", one=1)
                                .to_broadcast([P, nch, P]),
                            op=mybir.AluOpType.is_equal,
                        )
                        for bl in range(nb):
                            b = b0 + bl
                            for c in range(int(Q[g, t])):
                                ci = bl * int(Q[g, t]) + c
                                first = t == 0 and c == 0
                                last = t == NBUCK - 1 and c == int(Q[g, t]) - 1
                                if layer == 1:
                                    nc.tensor.matmul(
                                        aggs_psum[bl][:],
                                        lhsT=xg[:, ci * elem : (ci + 1) * elem],
                                        rhs=mb[:, ci * P : (ci + 1) * P],
                                        start=first, stop=last,
                                    )
                                else:
                                    nc.tensor.matmul(
                                        aggs_psum[bl][:],
                                        lhsT=mb[:, ci * P : (ci + 1) * P],
                                        rhs=xg[:, ci * elem : ci * elem + OUT],
                                        start=first, stop=last,
                                    )
                        ch_base += nch
                        sl_base += ni
                    ch0 += nb * qsum
                    slot0 += nb * qsum * P
                    # epilogue per block
                    for bl in range(nb):
                        b = b0 + bl
                        rows = P if b < NBLK - 1 else LAST_ROWS
                        if layer == 1:
                            aggs = wp.tile([P, P], f32, tag="aggs")
                            nc.vector.tensor_tensor(
                                out=aggs[:], in0=aggs_psum[bl][:],
                                in1=ninT_sb[:, b * P : (b + 1) * P],
                                op=mybir.AluOpType.mult,
                            )
                            x1a = wp.tile([P, P], f32, tag="x1a")
                            x1b = wp.tile([P, P], f32, tag="x1b")
                            for h, xt in ((0, x1a), (1, x1b)):
                                pz = pzp.tile([P, P], f32, tag="pz")
                                nc.tensor.matmul(
                                    pz[:], lhsT=w1_sb[:, h * P : (h + 1) * P],
                                    rhs=aggs[:], start=True, stop=True,
                                )
                                nc.scalar.activation(
                                    out=xt[:], in_=pz[:],
                                    func=mybir.ActivationFunctionType.Relu,
                                    bias=b1_sb[:, h : h + 1], scale=1.0,
                                )
                            pp = ppp.tile([P, OUT], f32, tag="pp")
                            nc.tensor.matmul(pp[:], lhsT=x1a[:],
                                             rhs=w2_sb[:, :OUT],
                                             start=True, stop=False)
                            nc.tensor.matmul(pp[:], lhsT=x1b[:],
                                             rhs=w2_sb[:, OUT:],
                                             start=False, stop=True)
                            p2s = wp.tile([P, 2 * OUT], bf16, tag="p2s")
                            nc.vector.tensor_scalar(
                                out=p2s[:, :OUT], in0=pp[:],
                                scalar1=noutcol_sb[:, b : b + 1], scalar2=None,
                                op0=mybir.AluOpType.mult,
                            )
                            nc.vector.memset(p2s[:, OUT:], 0.0)
                            nc.sync.dma_start(
                                out=p2_shard[b * P : b * P + rows, :],
                                in_=p2s[:rows, :])
                        else:
                            outs = wp.tile([P, OUT], f32, tag="outs")
                            nc.vector.tensor_scalar(
                                out=outs[:], in0=aggs_psum[bl][:],
                                scalar1=nincol_sb[:, b : b + 1], scalar2=None,
                                op0=mybir.AluOpType.mult,
                            )
                            nc.vector.tensor_tensor(
                                out=outs[:], in0=outs[:], in1=b2_sb[:],
                                op=mybir.AluOpType.add,
                            )
                            nc.sync.dma_start(
                                out=out_d[b * P : b * P + rows, :],
                                in_=outs[:rows, :])

            scatter_layer(1)

            nc.gpsimd.collective_compute(
                "AllGather",
                mybir.AluOpType.bypass,
                replica_groups=[list(range(NCORES))],
                ins=[p2_shard[:].opt()],
                outs=[p2_full[:].opt()],
            )

            scatter_layer(2)

    nc.finalize()
    return nc


def run_on_device(in_maps, Q, trace=False):
    from concourse.bass_utils import run_bass_kernel_spmd

    nc = _build_program(Q)
    return run_bass_kernel_spmd(nc, in_maps, core_ids=list(range(NCORES)),
                                trace=trace)


def kernel(h, src, dst, W1, b1, W2, b2):
    h = np.asarray(h, dtype=np.float32)
    src = np.asarray(src, dtype=np.int32)
    dst = np.asarray(dst, dtype=np.int32)
    W1 = np.asarray(W1, dtype=np.float32)
    b1 = np.asarray(b1, dtype=np.float32)
    W2 = np.asarray(W2, dtype=np.float32)
    b2 = np.asarray(b2, dtype=np.float32)

    Q, total_chunks, in_maps = _host_prep(h, src, dst, W1, b1, W2, b2)
    res = run_on_device(in_maps, Q)
    shards = [r["out"].astype(np.float32) for r in res.results]
    return np.concatenate(shards, axis=0)



# revision 3
# speedup vs baseline: 19.5304x; 19.5304x over previous
"""Distributed 2-layer GCN on 8 Trainium2 cores — pipelined redesign.

Key structure (dst-node 1D partitioning, core k owns dst rows [k*12500,(k+1)*12500)):
  - Layer 1: per group of 4 dst-blocks x 4 src-buckets: one dma_gather of
    hpre[src] rows (bf16, 256B), one-hot mask via iota==dstloc on DVE,
    per-chunk matmuls accumulate aggT[feat,dst] in PSUM. Merged group
    epilogue: *nin, @W1 (moving dim 512), +b1, relu, @W2, *nout -> p2 shard.
  - AllGather is split into 4 slices with Shared outputs; layer-2 src-buckets
    are aligned to AG slices, so bucket-t layer-2 work starts as soon as
    AG_t lands (overlaps layer-1 tail).
  - Layer 2: bucket-outer loop, gathers p2[src] rows from the slice tables,
    one-hot matmuls into PSUM [dst,64], accumulated across buckets in SBUF.
    Final per-block epilogue: *nin + b2 -> out.
  - 4 SWDGE queues round-robin for gathers; 32KB dynamic DMA scratch.
"""

import os
import numpy as np
import ml_dtypes

N, E, IN, HID, OUT = 100000, 1600000, 128, 256, 64
NCORES = 8
NLOC = N // NCORES            # 12500
P = 128
NBLK = (NLOC + P - 1) // P    # 98
LAST_ROWS = NLOC - (NBLK - 1) * P  # 84
BF16 = ml_dtypes.bfloat16
NBUCK = 4
BUCK1 = 25000                 # layer-1 src bucket size (int16-safe)
GB = 4                        # dst-blocks per group
NGRP = -(-NBLK // GB)         # 25
SUBMAX = 28                   # max chunks per dma_gather
NQ = 4                        # SWDGE queues
SCRATCH = 32768               # dynamic DMA scratch bytes/partition

# AG slices: groups per slice, local-row ranges, bucket table sizes
NSLICE = 4
SLICE_GRPS = [7, 7, 7, 4] if NSLICE == 4 else [25]
_rows = [g * GB * P for g in SLICE_GRPS]
R = [0]
for s in range(NSLICE):
    R.append(min(R[-1] + _rows[s], NLOC))
# R = [0, 3072, 6144, 9216, 12500]
SL_SZ = [R[s + 1] - R[s] for s in range(NSLICE)]
TAB_SZ = ([NCORES * sz for sz in SL_SZ] if NSLICE == 4
          else [25000, 25000, 25000, 25000])



def _host_prep(h, src, dst, W1, b1, W2, b2):
    deg_in = np.bincount(dst, minlength=N)
    deg_out = np.bincount(src, minlength=N)
    nin = (np.clip(deg_in, 1.0, None) ** -0.5).astype(np.float32)
    nout = (np.clip(deg_out, 1.0, None) ** -0.5).astype(np.float32)

    hpre = (h.astype(np.float32) * nout[:, None]).astype(BF16)

    # layer-2 bucket/index: node n -> slice s (by local row), idx within table
    if NSLICE == 4:
        k_all = np.arange(N, dtype=np.int64) // NLOC
        r_all = np.arange(N, dtype=np.int64) % NLOC
        s_all = np.searchsorted(np.asarray(R[1:]), r_all, side="right")
        s_all = np.clip(s_all, 0, 3)
        rp_all = r_all - np.asarray(R)[s_all]
        idx2_all = k_all * np.asarray(SL_SZ)[s_all] + rp_all
    else:
        s_all = np.arange(N, dtype=np.int64) // 25000
        idx2_all = np.arange(N, dtype=np.int64) % 25000

    def bucket_prep(bucket_of_src, idx_of_src, nbuck):
        """Per core: sort edges by (dst block, bucket); counts + arrays."""
        counts = np.zeros((NCORES, NBLK, nbuck), np.int64)
        edges = []
        for k in range(NCORES):
            sel = (dst // NLOC) == k
            es = src[sel].astype(np.int64)
            ed = (dst[sel] - k * NLOC).astype(np.int64)
            bk = bucket_of_src[es]
            key = (ed // P) * nbuck + bk
            order = np.argsort(key, kind="stable")
            es, ed = es[order], ed[order]
            c = np.bincount(key[order], minlength=NBLK * nbuck)
            counts[k] = c.reshape(NBLK, nbuck)
            edges.append((idx_of_src[es], ed))
        return counts, edges

    b1of = (np.arange(N, dtype=np.int64) // BUCK1)
    i1of = (np.arange(N, dtype=np.int64) % BUCK1)
    counts1, edges1 = bucket_prep(b1of, i1of, NBUCK)
    counts2, edges2 = bucket_prep(s_all, idx2_all, NBUCK)

    grp_nb = [min(GB, NBLK - g * GB) for g in range(NGRP)]

    def quotas(counts):
        Q = np.zeros((NGRP, NBUCK), np.int64)
        for g in range(NGRP):
            b0, b1_ = g * GB, min((g + 1) * GB, NBLK)
            Q[g] = np.maximum(1, -(-counts[:, b0:b1_, :].max(axis=(0, 1)) // P))
        return Q

    Q1, Q2 = quotas(counts1), quotas(counts2)

    def layout(Q, counts, edges, order_tmajor):
        """Build idx + dstloc arrays for the fixed SPMD chunk layout."""
        total_chunks = int(sum(grp_nb[g] * Q[g].sum() for g in range(NGRP)))
        TOT = total_chunks * P
        # slot offset for each (g,t) slab
        slab_off = {}
        pos = 0
        if order_tmajor:
            it = [(t, g) for t in range(NBUCK) for g in range(NGRP)]
        else:
            it = [(g, t) for g in range(NGRP) for t in range(NBUCK)]
        for a, b in it:
            g, t = (b, a) if order_tmajor else (a, b)
            slab_off[(g, t)] = pos
            pos += grp_nb[g] * int(Q[g, t]) * P
        assert pos == TOT
        idxs, dstlocs = [], []
        for k in range(NCORES):
            es, ed = edges[k]
            idx = np.zeros(TOT, np.int32)
            dstloc = np.full(TOT, 999.0, np.float32)
            starts = np.concatenate(
                [[0], np.cumsum(counts[k].reshape(-1))]).astype(np.int64)
            for g in range(NGRP):
                for t in range(NBUCK):
                    base = slab_off[(g, t)]
                    for bl in range(grp_nb[g]):
                        bb = g * GB + bl
                        n_bt = int(counts[k, bb, t])
                        s0 = int(starts[bb * NBUCK + t])
                        slot = base + bl * int(Q[g, t]) * P
                        idx[slot: slot + n_bt] = es[s0: s0 + n_bt]
                        dstloc[slot: slot + n_bt] = ed[s0: s0 + n_bt] % P
            wrap = np.tile(idx.astype(np.int16).reshape(-1, 16).T, (8, 1))
            idxs.append(np.ascontiguousarray(wrap))
            dstlocs.append(np.ascontiguousarray(
                dstloc.reshape(-1, P).T).astype(BF16))
        return total_chunks, idxs, dstlocs

    tc1, idx1s, dl1s = layout(Q1, counts1, edges1, order_tmajor=False)
    tc2, idx2s, dl2s = layout(Q2, counts2, edges2, order_tmajor=True)

    in_maps = []
    for k in range(NCORES):
        nin_loc = nin[k * NLOC: (k + 1) * NLOC]
        nout_loc = nout[k * NLOC: (k + 1) * NLOC]
        pad = NBLK * P - NLOC
        nin_cols = np.ascontiguousarray(
            np.pad(nin_loc, (0, pad)).reshape(NBLK, P).T, dtype=np.float32)
        nout_cols = np.ascontiguousarray(
            np.pad(nout_loc, (0, pad)).reshape(NBLK, P).T, dtype=np.float32)
        nin_tiled = np.tile(np.pad(nin_loc, (0, pad)),
                            (P, 1)).astype(BF16)
        in_maps.append({
            "hpre": hpre,
            "idx1": idx1s[k], "idx2": idx2s[k],
            "dstloc1": dl1s[k], "dstloc2": dl2s[k],
            "iota": np.tile(np.arange(P, dtype=np.float32),
                            (P, 1)).astype(BF16),
            "w1": np.ascontiguousarray(W1, dtype=np.float32),
            "w2p": np.ascontiguousarray(
                W2.reshape(2, P, OUT).transpose(1, 0, 2).reshape(P, 2 * OUT),
                dtype=np.float32),
            "b1p": np.ascontiguousarray(b1.reshape(2, P).T, dtype=np.float32),
            "b2bc": np.tile(b2.astype(np.float32), (P, 1)),
            "ninT": nin_tiled,
            "nin_cols": nin_cols,
            "nout_cols": nout_cols,
        })
    return Q1, Q2, tc1, tc2, in_maps


def _build_program(Q1, Q2):
    import concourse.bacc as bacc
    import concourse.mybir as mybir
    import concourse.tile as tile

    f32 = mybir.dt.float32
    bf16 = mybir.dt.bfloat16
    i16 = mybir.dt.int16

    grp_nb = [min(GB, NBLK - g * GB) for g in range(NGRP)]
    tc1 = int(sum(grp_nb[g] * Q1[g].sum() for g in range(NGRP)))
    tc2 = int(sum(grp_nb[g] * Q2[g].sum() for g in range(NGRP)))
    TOT1, TOT2 = tc1 * P, tc2 * P
    max_slab = GB * int(max(Q1.max(), Q2.max()))

    nc = bacc.Bacc(None, num_swdge_queues=NQ,
                   dynamic_dma_scratch_size=SCRATCH)
    qn = [0]

    def next_q():
        qn[0] = (qn[0] + 1) % NQ
        return qn[0]

    def subsplit(nch):
        nsub = -(-nch // SUBMAX)
        base = nch // nsub
        rem = nch - base * nsub
        return [base + (1 if i < rem else 0) for i in range(nsub)]

    sizes = set()
    for Q in (Q1, Q2):
        for g in range(NGRP):
            for t in range(NBUCK):
                for s in subsplit(grp_nb[g] * int(Q[g, t])):
                    sizes.add(s * P)
    size_regs = {s: nc.gpsimd.to_reg(s) for s in sorted(sizes)}

    hpre_d = nc.dram_tensor("hpre", [N, IN], bf16, kind="ExternalInput")
    idx1_d = nc.dram_tensor("idx1", [P, TOT1 // 16], i16, kind="ExternalInput")
    idx2_d = nc.dram_tensor("idx2", [P, TOT2 // 16], i16, kind="ExternalInput")
    dl1_d = nc.dram_tensor("dstloc1", [P, tc1], bf16, kind="ExternalInput")
    dl2_d = nc.dram_tensor("dstloc2", [P, tc2], bf16, kind="ExternalInput")
    iota_d = nc.dram_tensor("iota", [P, P], bf16, kind="ExternalInput")
    w1_d = nc.dram_tensor("w1", [IN, HID], f32, kind="ExternalInput")
    w2p_d = nc.dram_tensor("w2p", [P, 2 * OUT], f32, kind="ExternalInput")
    b1p_d = nc.dram_tensor("b1p", [P, 2], f32, kind="ExternalInput")
    b2bc_d = nc.dram_tensor("b2bc", [P, OUT], f32, kind="ExternalInput")
    ninT_d = nc.dram_tensor("ninT", [P, NBLK * P], bf16, kind="ExternalInput")
    nincol_d = nc.dram_tensor("nin_cols", [P, NBLK], f32, kind="ExternalInput")
    noutcol_d = nc.dram_tensor("nout_cols", [P, NBLK], f32, kind="ExternalInput")
    out_d = nc.dram_tensor("out", [NLOC, OUT], f32, kind="ExternalOutput")

    with tile.TileContext(nc) as tc:
        with (
            tc.tile_pool(name="const", bufs=1) as constp,
            tc.tile_pool(name="dram", bufs=1, space="DRAM") as dramp,
            tc.tile_pool(name="xg", bufs=5) as xgp,
            tc.tile_pool(name="mblk", bufs=3) as mp,
            tc.tile_pool(name="work", bufs=2) as wp,
            tc.tile_pool(name="pa", bufs=GB, space="PSUM") as pap,
            tc.tile_pool(name="pz", bufs=1, space="PSUM") as pzp,
            tc.tile_pool(name="pp", bufs=1, space="PSUM") as ppp,
            tc.tile_pool(name="pl2", bufs=2, space="PSUM") as pl2p,
        ):
            idx1_sb = constp.tile([P, TOT1 // 16], i16, tag="idx1")
            nc.sync.dma_start(out=idx1_sb[:], in_=idx1_d[:])
            idx2_sb = constp.tile([P, TOT2 // 16], i16, tag="idx2")
            nc.sync.dma_start(out=idx2_sb[:], in_=idx2_d[:])
            dl1_sb = constp.tile([P, tc1], bf16, tag="dl1")
            nc.sync.dma_start(out=dl1_sb[:], in_=dl1_d[:])
            dl2_sb = constp.tile([P, tc2], bf16, tag="dl2")
            nc.sync.dma_start(out=dl2_sb[:], in_=dl2_d[:])
            iotaw_sb = constp.tile([P, max_slab * P], bf16, tag="iotaw")
            for c in range(max_slab):
                nc.sync.dma_start(out=iotaw_sb[:, c * P: (c + 1) * P],
                                  in_=iota_d[:])
            w1_sb = constp.tile([IN, HID], f32, tag="w1")
            nc.sync.dma_start(out=w1_sb[:], in_=w1_d[:])
            w2_sb = constp.tile([P, 2 * OUT], f32, tag="w2")
            nc.sync.dma_start(out=w2_sb[:], in_=w2p_d[:])
            b1_sb = constp.tile([P, 2], f32, tag="b1")
            nc.sync.dma_start(out=b1_sb[:], in_=b1p_d[:])
            b2_sb = constp.tile([P, OUT], f32, tag="b2")
            nc.sync.dma_start(out=b2_sb[:], in_=b2bc_d[:])
            ninT_sb = constp.tile([P, NBLK * P], bf16, tag="ninT")
            nc.sync.dma_start(out=ninT_sb[:], in_=ninT_d[:])
            nincol_sb = constp.tile([P, NBLK], f32, tag="nincol")
            nc.sync.dma_start(out=nincol_sb[:], in_=nincol_d[:])
            noutcol_sb = constp.tile([P, NBLK], f32, tag="noutcol")
            nc.sync.dma_start(out=noutcol_sb[:], in_=noutcol_d[:])
            acc_sb = constp.tile([P, NBLK * OUT], f32, tag="acc")

            p2s_sl = [dramp.tile([SL_SZ[s], 2 * OUT], bf16, tag=f"p2s{s}",
                                 name=f"p2s{s}") for s in range(NSLICE)]
            if NSLICE == 4:
                p2f_sl = [dramp.tile([TAB_SZ[s], 2 * OUT], bf16, tag=f"p2f{s}",
                                     name=f"p2f{s}", addr_space="Shared")
                          for s in range(4)]
            else:
                p2f_one = dramp.tile([N, 2 * OUT], bf16, tag="p2f",
                                     name="p2f", addr_space="Shared")

            # ---- layer-1 slab offsets (g-major) and layer-2 (t-major) ----
            off1 = {}
            pos = 0
            for g in range(NGRP):
                for t in range(NBUCK):
                    off1[(g, t)] = pos
                    pos += grp_nb[g] * int(Q1[g, t])
            off2 = {}
            pos = 0
            for t in range(NBUCK):
                for g in range(NGRP):
                    off2[(g, t)] = pos
                    pos += grp_nb[g] * int(Q2[g, t])

            def gather_slab(table_ap, idx_sb, ch0, nch, elem):
                xg = xgp.tile([P, max_slab * IN], bf16, tag="xg", name="xg")
                sub0 = 0
                for snch in subsplit(nch):
                    sni = snch * P
                    sb0 = (ch0 + sub0) * P
                    nc.gpsimd.dma_gather(
                        out_ap=xg[:, sub0 * elem: (sub0 + snch) * elem]
                            .rearrange("p (c d) -> p c d", d=elem),
                        in_ap=table_ap,
                        idxs_ap=idx_sb[:, sb0 // 16: (sb0 + sni) // 16],
                        num_idxs=sni,
                        num_idxs_reg=size_regs[sni],
                        elem_size=elem,
                        single_packet=False,
                        queue_num=next_q(),
                    )
                    sub0 += snch
                return xg

            def mask_slab(dl_sb, ch0, nch):
                mb = mp.tile([P, max_slab * P], bf16, tag="m", name="m")
                nc.vector.tensor_tensor(
                    out=mb[:, : nch * P].rearrange("p (c d) -> p c d", d=P),
                    in0=iotaw_sb[:, : nch * P].rearrange(
                        "p (c d) -> p c d", d=P),
                    in1=dl_sb[:, ch0: ch0 + nch]
                        .rearrange("p (c one) -> p c one", one=1)
                        .to_broadcast([P, nch, P]),
                    op=mybir.AluOpType.is_equal,
                )
                return mb

            # ================= layer 1 =================
            g0 = 0
            for s in range(NSLICE):
                for g in range(g0, g0 + SLICE_GRPS[s]):
                    nb = grp_nb[g]
                    b0 = g * GB
                    aggs_psum = [
                        pap.tile([P, P], f32, tag="pa",
                                 name=f"pa1_{g}_{bl}")
                        for bl in range(nb)
                    ]
                    for t in range(NBUCK):
                        q = int(Q1[g, t])
                        nch = nb * q
                        ch0 = off1[(g, t)]
                        xg = gather_slab(
                            hpre_d[t * BUCK1: (t + 1) * BUCK1, :],
                            idx1_sb, ch0, nch, IN)
                        mb = mask_slab(dl1_sb, ch0, nch)
                        for bl in range(nb):
                            for c in range(q):
                                ci = bl * q + c
                                nc.tensor.matmul(
                                    aggs_psum[bl][:],
                                    lhsT=xg[:, ci * IN: (ci + 1) * IN],
                                    rhs=mb[:, ci * P: (ci + 1) * P],
                                    start=(t == 0 and c == 0),
                                    stop=(t == NBUCK - 1 and c == q - 1),
                                )
                    # merged group epilogue
                    w = nb * P
                    aggs4 = wp.tile([P, GB * P], f32, tag="aggs4")
                    for bl in range(nb):
                        b = b0 + bl
                        nc.vector.tensor_tensor(
                            out=aggs4[:, bl * P: (bl + 1) * P],
                            in0=aggs_psum[bl][:],
                            in1=ninT_sb[:, b * P: (b + 1) * P],
                            op=mybir.AluOpType.mult,
                        )
                    x1a = wp.tile([P, GB * P], f32, tag="x1a")
                    x1b = wp.tile([P, GB * P], f32, tag="x1b")
                    for hh, xt in ((0, x1a), (1, x1b)):
                        pz = pzp.tile([P, GB * P], f32, tag="pz")
                        nc.tensor.matmul(
                            pz[:, :w], lhsT=w1_sb[:, hh * P: (hh + 1) * P],
                            rhs=aggs4[:, :w], start=True, stop=True,
                        )
                        nc.scalar.activation(
                            out=xt[:, :w], in_=pz[:, :w],
                            func=mybir.ActivationFunctionType.Relu,
                            bias=b1_sb[:, hh: hh + 1], scale=1.0,
                        )
                    for bl in range(nb):
                        b = b0 + bl
                        rows = P if b < NBLK - 1 else LAST_ROWS
                        pp = ppp.tile([P, OUT], f32, tag="pp")
                        nc.tensor.matmul(pp[:], lhsT=x1a[:, bl * P: (bl + 1) * P],
                                         rhs=w2_sb[:, :OUT],
                                         start=True, stop=False)
                        nc.tensor.matmul(pp[:], lhsT=x1b[:, bl * P: (bl + 1) * P],
                                         rhs=w2_sb[:, OUT:],
                                         start=False, stop=True)
                        p2s = wp.tile([P, 2 * OUT], bf16, tag="p2s")
                        nc.vector.tensor_scalar(
                            out=p2s[:, :OUT], in0=pp[:],
                            scalar1=noutcol_sb[:, b: b + 1], scalar2=None,
                            op0=mybir.AluOpType.mult,
                        )
                        nc.vector.memset(p2s[:, OUT:], 0.0)
                        row0 = b * P - R[s]
                        nc.sync.dma_start(
                            out=p2s_sl[s][row0: row0 + rows, :],
                            in_=p2s[:rows, :])
                g0 += SLICE_GRPS[s]
                nc.gpsimd.collective_compute(
                    "AllGather",
                    mybir.AluOpType.bypass,
                    replica_groups=[list(range(NCORES))],
                    ins=[p2s_sl[s][:].opt()],
                    outs=[(p2f_sl[s] if NSLICE == 4 else p2f_one)[:].opt()],
                )

            # ================= layer 2 =================
            for t in range(NBUCK):
                for g in range(NGRP):
                    nb = grp_nb[g]
                    b0 = g * GB
                    q = int(Q2[g, t])
                    nch = nb * q
                    ch0 = off2[(g, t)]
                    tab2 = (p2f_sl[t][:] if NSLICE == 4
                            else p2f_one[t * 25000: (t + 1) * 25000, :])
                    xg = gather_slab(tab2, idx2_sb, ch0, nch, 2 * OUT)
                    mb = mask_slab(dl2_sb, ch0, nch)
                    for bl in range(nb):
                        b = b0 + bl
                        ps = pl2p.tile([P, OUT], f32, tag="pl2",
                                       name=f"pl2_{t}_{g}_{bl}")
                        for c in range(q):
                            ci = bl * q + c
                            nc.tensor.matmul(
                                ps[:],
                                lhsT=mb[:, ci * P: (ci + 1) * P],
                                rhs=xg[:, ci * 2 * OUT: ci * 2 * OUT + OUT],
                                start=(c == 0), stop=(c == q - 1),
                            )
                        if t == 0:
                            nc.scalar.copy(
                                out=acc_sb[:, b * OUT: (b + 1) * OUT],
                                in_=ps[:],
                            )
                        else:
                            nc.vector.tensor_tensor(
                                out=acc_sb[:, b * OUT: (b + 1) * OUT],
                                in0=acc_sb[:, b * OUT: (b + 1) * OUT],
                                in1=ps[:],
                                op=mybir.AluOpType.add,
                            )
                        if t == NBUCK - 1:
                            rows = P if b < NBLK - 1 else LAST_ROWS
                            outs = wp.tile([P, OUT], f32, tag="outs")
                            nc.vector.tensor_scalar(
                                out=outs[:],
                                in0=acc_sb[:, b * OUT: (b + 1) * OUT],
                                scalar1=nincol_sb[:, b: b + 1], scalar2=None,
                                op0=mybir.AluOpType.mult,
                            )
                            nc.vector.tensor_tensor(
                                out=outs[:], in0=outs[:], in1=b2_sb[:],
                                op=mybir.AluOpType.add,
                            )
                            nc.sync.dma_start(
                                out=out_d[b * P: b * P + rows, :],
                                in_=outs[:rows, :])

    nc.finalize()
    return nc


def kernel(h, src, dst, W1, b1, W2, b2):
    h = np.asarray(h, dtype=np.float32)
    src = np.asarray(src, dtype=np.int32)
    dst = np.asarray(dst, dtype=np.int32)
    W1 = np.asarray(W1, dtype=np.float32)
    b1 = np.asarray(b1, dtype=np.float32)
    W2 = np.asarray(W2, dtype=np.float32)
    b2 = np.asarray(b2, dtype=np.float32)

    Q1, Q2, tc1, tc2, in_maps = _host_prep(h, src, dst, W1, b1, W2, b2)
    from concourse.bass_utils import run_bass_kernel_spmd
    nc = _build_program(Q1, Q2)
    res = run_bass_kernel_spmd(nc, in_maps, core_ids=list(range(NCORES)))
    shards = [r["out"].astype(np.float32) for r in res.results]
    return np.concatenate(shards, axis=0)


# revision 4
# speedup vs baseline: 21.5410x; 1.1029x over previous
"""Distributed 2-layer GCN on 8 Trainium2 cores — pipelined redesign.

Key structure (dst-node 1D partitioning, core k owns dst rows [k*12500,(k+1)*12500)):
  - Layer 1: per group of 4 dst-blocks x 4 src-buckets: one dma_gather of
    hpre[src] rows (bf16, 256B), one-hot mask via iota==dstloc on DVE,
    per-chunk matmuls accumulate aggT[feat,dst] in PSUM. Merged group
    epilogue: *nin, @W1 (moving dim 512), +b1, relu, @W2, *nout -> p2 shard.
  - AllGather is split into 4 slices with Shared outputs; layer-2 src-buckets
    are aligned to AG slices, so bucket-t layer-2 work starts as soon as
    AG_t lands (overlaps layer-1 tail).
  - Layer 2: bucket-outer loop, gathers p2[src] rows from the slice tables,
    one-hot matmuls into PSUM [dst,64], accumulated across buckets in SBUF.
    Final per-block epilogue: *nin + b2 -> out.
  - 4 SWDGE queues round-robin for gathers; 32KB dynamic DMA scratch.
"""

import os
import numpy as np
import ml_dtypes

N, E, IN, HID, OUT = 100000, 1600000, 128, 256, 64
NCORES = 8
NLOC = N // NCORES            # 12500
P = 128
NBLK = (NLOC + P - 1) // P    # 98
LAST_ROWS = NLOC - (NBLK - 1) * P  # 84
BF16 = ml_dtypes.bfloat16
NBUCK = 4
BUCK1 = 25000                 # layer-1 src bucket size (int16-safe)
GB = 4                        # dst-blocks per group
NGRP = -(-NBLK // GB)         # 25
SUBMAX = 28                   # max chunks per dma_gather
NQ = 4                        # SWDGE queues
SCRATCH = 32768               # dynamic DMA scratch bytes/partition

# AG slices: groups per slice, local-row ranges, bucket table sizes
NSLICE = 4
SLICE_GRPS = [7, 7, 7, 4] if NSLICE == 4 else [25]
_rows = [g * GB * P for g in SLICE_GRPS]
R = [0]
for s in range(NSLICE):
    R.append(min(R[-1] + _rows[s], NLOC))
# R = [0, 3072, 6144, 9216, 12500]
SL_SZ = [R[s + 1] - R[s] for s in range(NSLICE)]
TAB_SZ = ([NCORES * sz for sz in SL_SZ] if NSLICE == 4
          else [25000, 25000, 25000, 25000])



def _host_prep(h, src, dst, W1, b1, W2, b2):
    deg_in = np.bincount(dst, minlength=N)
    deg_out = np.bincount(src, minlength=N)
    nin = (np.clip(deg_in, 1.0, None) ** -0.5).astype(np.float32)
    nout = (np.clip(deg_out, 1.0, None) ** -0.5).astype(np.float32)

    hpre = (h.astype(np.float32) * nout[:, None]).astype(BF16)

    # layer-2 bucket/index: node n -> slice s (by local row), idx within table
    if NSLICE == 4:
        k_all = np.arange(N, dtype=np.int64) // NLOC
        r_all = np.arange(N, dtype=np.int64) % NLOC
        s_all = np.searchsorted(np.asarray(R[1:]), r_all, side="right")
        s_all = np.clip(s_all, 0, 3)
        rp_all = r_all - np.asarray(R)[s_all]
        idx2_all = k_all * np.asarray(SL_SZ)[s_all] + rp_all
    else:
        s_all = np.arange(N, dtype=np.int64) // 25000
        idx2_all = np.arange(N, dtype=np.int64) % 25000

    def bucket_prep(bucket_of_src, idx_of_src, nbuck):
        """Per core: sort edges by (dst block, bucket); counts + arrays."""
        counts = np.zeros((NCORES, NBLK, nbuck), np.int64)
        edges = []
        for k in range(NCORES):
            sel = (dst // NLOC) == k
            es = src[sel].astype(np.int64)
            ed = (dst[sel] - k * NLOC).astype(np.int64)
            bk = bucket_of_src[es]
            key = (ed // P) * nbuck + bk
            order = np.argsort(key, kind="stable")
            es, ed = es[order], ed[order]
            c = np.bincount(key[order], minlength=NBLK * nbuck)
            counts[k] = c.reshape(NBLK, nbuck)
            edges.append((idx_of_src[es], ed))
        return counts, edges

    b1of = (np.arange(N, dtype=np.int64) // BUCK1)
    i1of = (np.arange(N, dtype=np.int64) % BUCK1)
    counts1, edges1 = bucket_prep(b1of, i1of, NBUCK)
    counts2, edges2 = bucket_prep(s_all, idx2_all, NBUCK)

    grp_nb = [min(GB, NBLK - g * GB) for g in range(NGRP)]

    def quotas(counts):
        Q = np.zeros((NGRP, NBUCK), np.int64)
        for g in range(NGRP):
            b0, b1_ = g * GB, min((g + 1) * GB, NBLK)
            Q[g] = np.maximum(1, -(-counts[:, b0:b1_, :].max(axis=(0, 1)) // P))
        return Q

    Q1, Q2 = quotas(counts1), quotas(counts2)

    def layout(Q, counts, edges, order_tmajor):
        """Build idx + dstloc arrays for the fixed SPMD chunk layout."""
        total_chunks = int(sum(grp_nb[g] * Q[g].sum() for g in range(NGRP)))
        TOT = total_chunks * P
        # slot offset for each (g,t) slab
        slab_off = {}
        pos = 0
        if order_tmajor:
            it = [(t, g) for t in range(NBUCK) for g in range(NGRP)]
        else:
            it = [(g, t) for g in range(NGRP) for t in range(NBUCK)]
        for a, b in it:
            g, t = (b, a) if order_tmajor else (a, b)
            slab_off[(g, t)] = pos
            pos += grp_nb[g] * int(Q[g, t]) * P
        assert pos == TOT
        idxs, dstlocs = [], []
        for k in range(NCORES):
            es, ed = edges[k]
            idx = np.zeros(TOT, np.int32)
            dstloc = np.full(TOT, 999.0, np.float32)
            starts = np.concatenate(
                [[0], np.cumsum(counts[k].reshape(-1))]).astype(np.int64)
            for g in range(NGRP):
                for t in range(NBUCK):
                    base = slab_off[(g, t)]
                    for bl in range(grp_nb[g]):
                        bb = g * GB + bl
                        n_bt = int(counts[k, bb, t])
                        s0 = int(starts[bb * NBUCK + t])
                        slot = base + bl * int(Q[g, t]) * P
                        idx[slot: slot + n_bt] = es[s0: s0 + n_bt]
                        dstloc[slot: slot + n_bt] = ed[s0: s0 + n_bt] % P
            wrap = np.tile(idx.astype(np.int16).reshape(-1, 16).T, (8, 1))
            idxs.append(np.ascontiguousarray(wrap))
            dstlocs.append(np.ascontiguousarray(
                dstloc.reshape(-1, P).T).astype(BF16))
        return total_chunks, idxs, dstlocs

    tc1, idx1s, dl1s = layout(Q1, counts1, edges1, order_tmajor=False)
    tc2, idx2s, dl2s = layout(Q2, counts2, edges2, order_tmajor=True)

    in_maps = []
    for k in range(NCORES):
        nin_loc = nin[k * NLOC: (k + 1) * NLOC]
        nout_loc = nout[k * NLOC: (k + 1) * NLOC]
        pad = NBLK * P - NLOC
        nin_cols = np.ascontiguousarray(
            np.pad(nin_loc, (0, pad)).reshape(NBLK, P).T, dtype=np.float32)
        nout_cols = np.ascontiguousarray(
            np.pad(nout_loc, (0, pad)).reshape(NBLK, P).T, dtype=np.float32)
        nin_tiled = np.tile(np.pad(nin_loc, (0, pad)),
                            (P, 1)).astype(BF16)
        in_maps.append({
            "hpre": hpre,
            "idx1": idx1s[k], "idx2": idx2s[k],
            "dstloc1": dl1s[k], "dstloc2": dl2s[k],
            "iota": np.tile(np.arange(P, dtype=np.float32),
                            (P, 1)).astype(BF16),
            "w1": np.ascontiguousarray(W1, dtype=np.float32),
            "w2p": np.ascontiguousarray(
                W2.reshape(2, P, OUT).transpose(1, 0, 2).reshape(P, 2 * OUT),
                dtype=np.float32),
            "b1p": np.ascontiguousarray(b1.reshape(2, P).T, dtype=np.float32),
            "b2bc": np.tile(b2.astype(np.float32), (P, 1)),
            "ninT": nin_tiled,
            "nin_cols": nin_cols,
            "nout_cols": nout_cols,
        })
    return Q1, Q2, tc1, tc2, in_maps


def _build_program(Q1, Q2):
    import concourse.bacc as bacc
    import concourse.mybir as mybir
    import concourse.tile as tile

    f32 = mybir.dt.float32
    bf16 = mybir.dt.bfloat16
    i16 = mybir.dt.int16

    grp_nb = [min(GB, NBLK - g * GB) for g in range(NGRP)]
    tc1 = int(sum(grp_nb[g] * Q1[g].sum() for g in range(NGRP)))
    tc2 = int(sum(grp_nb[g] * Q2[g].sum() for g in range(NGRP)))
    TOT1, TOT2 = tc1 * P, tc2 * P
    max_slab = GB * int(max(Q1.max(), Q2.max()))

    nc = bacc.Bacc(None, num_swdge_queues=NQ,
                   dynamic_dma_scratch_size=SCRATCH)
    qn = [0]

    def next_q():
        qn[0] = (qn[0] + 1) % NQ
        return qn[0]

    def subsplit(nch):
        nsub = -(-nch // SUBMAX)
        base = nch // nsub
        rem = nch - base * nsub
        return [base + (1 if i < rem else 0) for i in range(nsub)]

    sizes = set()
    for Q in (Q1, Q2):
        for g in range(NGRP):
            for t in range(NBUCK):
                for s in subsplit(grp_nb[g] * int(Q[g, t])):
                    sizes.add(s * P)
    size_regs = {s: nc.gpsimd.to_reg(s) for s in sorted(sizes)}

    hpre_d = nc.dram_tensor("hpre", [N, IN], bf16, kind="ExternalInput")
    idx1_d = nc.dram_tensor("idx1", [P, TOT1 // 16], i16, kind="ExternalInput")
    idx2_d = nc.dram_tensor("idx2", [P, TOT2 // 16], i16, kind="ExternalInput")
    dl1_d = nc.dram_tensor("dstloc1", [P, tc1], bf16, kind="ExternalInput")
    dl2_d = nc.dram_tensor("dstloc2", [P, tc2], bf16, kind="ExternalInput")
    iota_d = nc.dram_tensor("iota", [P, P], bf16, kind="ExternalInput")
    w1_d = nc.dram_tensor("w1", [IN, HID], f32, kind="ExternalInput")
    w2p_d = nc.dram_tensor("w2p", [P, 2 * OUT], f32, kind="ExternalInput")
    b1p_d = nc.dram_tensor("b1p", [P, 2], f32, kind="ExternalInput")
    b2bc_d = nc.dram_tensor("b2bc", [P, OUT], f32, kind="ExternalInput")
    ninT_d = nc.dram_tensor("ninT", [P, NBLK * P], bf16, kind="ExternalInput")
    nincol_d = nc.dram_tensor("nin_cols", [P, NBLK], f32, kind="ExternalInput")
    noutcol_d = nc.dram_tensor("nout_cols", [P, NBLK], f32, kind="ExternalInput")
    out_d = nc.dram_tensor("out", [NLOC, OUT], f32, kind="ExternalOutput")

    with tile.TileContext(nc) as tc:
        with (
            tc.tile_pool(name="const", bufs=1) as constp,
            tc.tile_pool(name="dram", bufs=1, space="DRAM") as dramp,
            tc.tile_pool(name="xg", bufs=5) as xgp,
            tc.tile_pool(name="mblk", bufs=3) as mp,
            tc.tile_pool(name="work", bufs=2) as wp,
            tc.tile_pool(name="pa", bufs=GB, space="PSUM") as pap,
            tc.tile_pool(name="pz", bufs=2, space="PSUM") as pzp,
            tc.tile_pool(name="pl2", bufs=2, space="PSUM") as pl2p,
        ):
            idx1_sb = constp.tile([P, TOT1 // 16], i16, tag="idx1")
            nc.sync.dma_start(out=idx1_sb[:], in_=idx1_d[:])
            idx2_sb = constp.tile([P, TOT2 // 16], i16, tag="idx2")
            nc.sync.dma_start(out=idx2_sb[:], in_=idx2_d[:])
            dl1_sb = constp.tile([P, tc1], bf16, tag="dl1")
            nc.sync.dma_start(out=dl1_sb[:], in_=dl1_d[:])
            dl2_sb = constp.tile([P, tc2], bf16, tag="dl2")
            nc.sync.dma_start(out=dl2_sb[:], in_=dl2_d[:])
            iotaw_sb = constp.tile([P, max_slab * P], bf16, tag="iotaw")
            for c in range(max_slab):
                nc.sync.dma_start(out=iotaw_sb[:, c * P: (c + 1) * P],
                                  in_=iota_d[:])
            w1_sb = constp.tile([IN, HID], f32, tag="w1")
            nc.sync.dma_start(out=w1_sb[:], in_=w1_d[:])
            w2_sb = constp.tile([P, 2 * OUT], f32, tag="w2")
            nc.sync.dma_start(out=w2_sb[:], in_=w2p_d[:])
            b1_sb = constp.tile([P, 2], f32, tag="b1")
            nc.sync.dma_start(out=b1_sb[:], in_=b1p_d[:])
            b2_sb = constp.tile([P, OUT], f32, tag="b2")
            nc.sync.dma_start(out=b2_sb[:], in_=b2bc_d[:])
            ninT_sb = constp.tile([P, NBLK * P], bf16, tag="ninT")
            nc.sync.dma_start(out=ninT_sb[:], in_=ninT_d[:])
            nincol_sb = constp.tile([P, NBLK], f32, tag="nincol")
            nc.sync.dma_start(out=nincol_sb[:], in_=nincol_d[:])
            noutcol_sb = constp.tile([P, NBLK], f32, tag="noutcol")
            nc.sync.dma_start(out=noutcol_sb[:], in_=noutcol_d[:])
            acc_sb = constp.tile([P, NBLK * OUT], f32, tag="acc")

            p2s_sl = [dramp.tile([SL_SZ[s], 2 * OUT], bf16, tag=f"p2s{s}",
                                 name=f"p2s{s}") for s in range(NSLICE)]
            if NSLICE == 4:
                p2f_sl = [dramp.tile([TAB_SZ[s], 2 * OUT], bf16, tag=f"p2f{s}",
                                     name=f"p2f{s}", addr_space="Shared")
                          for s in range(4)]
            else:
                p2f_one = dramp.tile([N, 2 * OUT], bf16, tag="p2f",
                                     name="p2f", addr_space="Shared")

            # ---- layer-1 slab offsets (g-major) and layer-2 (t-major) ----
            off1 = {}
            pos = 0
            for g in range(NGRP):
                for t in range(NBUCK):
                    off1[(g, t)] = pos
                    pos += grp_nb[g] * int(Q1[g, t])
            off2 = {}
            pos = 0
            for t in range(NBUCK):
                for g in range(NGRP):
                    off2[(g, t)] = pos
                    pos += grp_nb[g] * int(Q2[g, t])

            def gather_slab(table_ap, idx_sb, ch0, nch, elem):
                xg = xgp.tile([P, max_slab * IN], bf16, tag="xg", name="xg")
                sub0 = 0
                for snch in subsplit(nch):
                    sni = snch * P
                    sb0 = (ch0 + sub0) * P
                    nc.gpsimd.dma_gather(
                        out_ap=xg[:, sub0 * elem: (sub0 + snch) * elem]
                            .rearrange("p (c d) -> p c d", d=elem),
                        in_ap=table_ap,
                        idxs_ap=idx_sb[:, sb0 // 16: (sb0 + sni) // 16],
                        num_idxs=sni,
                        num_idxs_reg=size_regs[sni],
                        elem_size=elem,
                        single_packet=False,
                        queue_num=next_q(),
                    )
                    sub0 += snch
                return xg

            def mask_slab(dl_sb, ch0, nch):
                mb = mp.tile([P, max_slab * P], bf16, tag="m", name="m")
                nc.vector.tensor_tensor(
                    out=mb[:, : nch * P].rearrange("p (c d) -> p c d", d=P),
                    in0=iotaw_sb[:, : nch * P].rearrange(
                        "p (c d) -> p c d", d=P),
                    in1=dl_sb[:, ch0: ch0 + nch]
                        .rearrange("p (c one) -> p c one", one=1)
                        .to_broadcast([P, nch, P]),
                    op=mybir.AluOpType.is_equal,
                )
                return mb

            # ================= layer 1 =================
            g0 = 0
            for s in range(NSLICE):
                for g in range(g0, g0 + SLICE_GRPS[s]):
                    nb = grp_nb[g]
                    b0 = g * GB
                    aggs_psum = [
                        pap.tile([P, P], f32, tag="pa",
                                 name=f"pa1_{g}_{bl}")
                        for bl in range(nb)
                    ]
                    for t in range(NBUCK):
                        q = int(Q1[g, t])
                        nch = nb * q
                        ch0 = off1[(g, t)]
                        xg = gather_slab(
                            hpre_d[t * BUCK1: (t + 1) * BUCK1, :],
                            idx1_sb, ch0, nch, IN)
                        mb = mask_slab(dl1_sb, ch0, nch)
                        for bl in range(nb):
                            for c in range(q):
                                ci = bl * q + c
                                nc.tensor.matmul(
                                    aggs_psum[bl][:],
                                    lhsT=xg[:, ci * IN: (ci + 1) * IN],
                                    rhs=mb[:, ci * P: (ci + 1) * P],
                                    start=(t == 0 and c == 0),
                                    stop=(t == NBUCK - 1 and c == q - 1),
                                )
                    # merged group epilogue
                    w = nb * P
                    aggs4 = wp.tile([P, GB * P], f32, tag="aggs4")
                    for bl in range(nb):
                        b = b0 + bl
                        nc.vector.tensor_tensor(
                            out=aggs4[:, bl * P: (bl + 1) * P],
                            in0=aggs_psum[bl][:],
                            in1=ninT_sb[:, b * P: (b + 1) * P],
                            op=mybir.AluOpType.mult,
                        )
                    x1a = wp.tile([P, GB * P], f32, tag="x1a")
                    x1b = wp.tile([P, GB * P], f32, tag="x1b")
                    for hh, xt in ((0, x1a), (1, x1b)):
                        pz = pzp.tile([P, GB * P], f32, tag="pz")
                        nc.tensor.matmul(
                            pz[:, :w], lhsT=w1_sb[:, hh * P: (hh + 1) * P],
                            rhs=aggs4[:, :w], start=True, stop=True,
                        )
                        nc.scalar.activation(
                            out=xt[:, :w], in_=pz[:, :w],
                            func=mybir.ActivationFunctionType.Relu,
                            bias=b1_sb[:, hh: hh + 1], scale=1.0,
                        )
                    for bl in range(nb):
                        b = b0 + bl
                        rows = P if b < NBLK - 1 else LAST_ROWS
                        pp = pzp.tile([P, OUT], f32, tag="pz")
                        nc.tensor.matmul(pp[:], lhsT=x1a[:, bl * P: (bl + 1) * P],
                                         rhs=w2_sb[:, :OUT],
                                         start=True, stop=False)
                        nc.tensor.matmul(pp[:], lhsT=x1b[:, bl * P: (bl + 1) * P],
                                         rhs=w2_sb[:, OUT:],
                                         start=False, stop=True)
                        p2s = wp.tile([P, 2 * OUT], bf16, tag="p2s")
                        nc.scalar.mul(out=p2s[:, :OUT], in_=pp[:],
                                      mul=noutcol_sb[:, b: b + 1])
                        nc.vector.memset(p2s[:, OUT:], 0.0)
                        row0 = b * P - R[s]
                        nc.sync.dma_start(
                            out=p2s_sl[s][row0: row0 + rows, :],
                            in_=p2s[:rows, :])
                g0 += SLICE_GRPS[s]
                nc.gpsimd.collective_compute(
                    "AllGather",
                    mybir.AluOpType.bypass,
                    replica_groups=[list(range(NCORES))],
                    ins=[p2s_sl[s][:].opt()],
                    outs=[(p2f_sl[s] if NSLICE == 4 else p2f_one)[:].opt()],
                )

            # ================= layer 2 =================
            for t in range(NBUCK):
                for g in range(NGRP):
                    nb = grp_nb[g]
                    b0 = g * GB
                    q = int(Q2[g, t])
                    nch = nb * q
                    ch0 = off2[(g, t)]
                    tab2 = (p2f_sl[t][:] if NSLICE == 4
                            else p2f_one[t * 25000: (t + 1) * 25000, :])
                    xg = gather_slab(tab2, idx2_sb, ch0, nch, 2 * OUT)
                    mb = mask_slab(dl2_sb, ch0, nch)
                    for bl in range(nb):
                        b = b0 + bl
                        ps = pl2p.tile([P, OUT], f32, tag="pl2",
                                       name=f"pl2_{t}_{g}_{bl}")
                        for c in range(q):
                            ci = bl * q + c
                            nc.tensor.matmul(
                                ps[:],
                                lhsT=mb[:, ci * P: (ci + 1) * P],
                                rhs=xg[:, ci * 2 * OUT: ci * 2 * OUT + OUT],
                                start=(c == 0), stop=(c == q - 1),
                            )
                        if t == 0:
                            nc.scalar.copy(
                                out=acc_sb[:, b * OUT: (b + 1) * OUT],
                                in_=ps[:],
                            )
                        else:
                            nc.vector.tensor_tensor(
                                out=acc_sb[:, b * OUT: (b + 1) * OUT],
                                in0=acc_sb[:, b * OUT: (b + 1) * OUT],
                                in1=ps[:],
                                op=mybir.AluOpType.add,
                            )
                        if t == NBUCK - 1:
                            rows = P if b < NBLK - 1 else LAST_ROWS
                            outs = wp.tile([P, OUT], f32, tag="outs")
                            nc.scalar.mul(
                                out=outs[:],
                                in_=acc_sb[:, b * OUT: (b + 1) * OUT],
                                mul=nincol_sb[:, b: b + 1])
                            nc.vector.tensor_tensor(
                                out=outs[:], in0=outs[:], in1=b2_sb[:],
                                op=mybir.AluOpType.add,
                            )
                            nc.sync.dma_start(
                                out=out_d[b * P: b * P + rows, :],
                                in_=outs[:rows, :])

    nc.finalize()
    return nc


def kernel(h, src, dst, W1, b1, W2, b2):
    h = np.asarray(h, dtype=np.float32)
    src = np.asarray(src, dtype=np.int32)
    dst = np.asarray(dst, dtype=np.int32)
    W1 = np.asarray(W1, dtype=np.float32)
    b1 = np.asarray(b1, dtype=np.float32)
    W2 = np.asarray(W2, dtype=np.float32)
    b2 = np.asarray(b2, dtype=np.float32)

    Q1, Q2, tc1, tc2, in_maps = _host_prep(h, src, dst, W1, b1, W2, b2)
    from concourse.bass_utils import run_bass_kernel_spmd
    nc = _build_program(Q1, Q2)
    res = run_bass_kernel_spmd(nc, in_maps, core_ids=list(range(NCORES)))
    shards = [r["out"].astype(np.float32) for r in res.results]
    return np.concatenate(shards, axis=0)
